# revision 7
# baseline (speedup 1.0000x reference)
"""Trainium2 Bass kernel for MergedQKVParallelLinearWithDelta.

out = x @ base_weight.T + per-token-indexed GPTQ-int4 delta matmul
(out[t] += x[t] @ Wdelta[indices[t]]).

Strategy:
- Tensor-parallel along the output dim N=6144 across 8 cores (768 cols
  each: q 512 + k 128 + v 128), x and indices replicated.
- Host: stable-sort tokens by delta index (MoE routing -> each token
  row is multiplied by exactly one delta, 4x fewer FLOPs than masking),
  transpose x to K-major, dequantize the int4 deltas to fp32 shards.
- Device: per 128-token tile, accumulate base + that tile's delta
  segment(s) into one PSUM bank over 32 K-chunks of float32r matmuls
  (1 cyc/row at N>=256 -> full 78.6 TFLOP/s rate). Output N is split
  into two 384-col halves so weights fit in SBUF.
- Host: concat core shards, unpermute token rows.
"""
import sys

if '/opt/trn_rl_repo' not in sys.path:
    sys.path.insert(0, '/opt/trn_rl_repo')

from contextlib import ExitStack

import numpy as np

import concourse.bass as bass
import concourse.tile as tile
from concourse import bacc, bass_utils, mybir

MAX_DELTAS = 4
PACK = 8
HIDDEN = 4096
Q_SLICE = 4096
KV_SLICE = 1024
TOKENS = 4096
NCORES = 8

QS = Q_SLICE // NCORES          # 512 q cols per core
KS = KV_SLICE // NCORES         # 128 k (and v) cols per core
NSH = QS + 2 * KS               # 768 cols per core
HALF = NSH // 2                 # 384
KC = HIDDEN // 128              # 32 K-chunks
TT = TOKENS // 128              # 32 token tiles

F32R = mybir.dt.float32r
F32 = mybir.dt.float32


def _plan(counts):
    """Pad each delta group to a multiple of 128 tokens so every token
    tile has exactly one delta (full-width matmuls only — PSUM row-offset
    matmuls are ISA-restricted). Returns (n_tiles, t_dev, segs, po)."""
    pc = [(int(c) + 127) // 128 * 128 for c in counts]
    po = np.concatenate([[0], np.cumsum(pc)])
    t_dev = int(po[-1])
    n_tiles = t_dev // 128
    segs = []
    for ti in range(n_tiles):
        t0 = ti * 128
        tile_segs = []
        for g in range(MAX_DELTAS):
            if int(po[g]) <= t0 < int(po[g]) + pc[g] and counts[g] > 0:
                tile_segs.append((g, 0, 128))
        segs.append(tile_segs)
    return n_tiles, t_dev, segs, po


_nc_cache = {}


def _build(n_tiles, segs_key):
    segs = [list(s) for s in segs_key]
    nc = bacc.Bacc("TRN2", target_bir_lowering=False, debug=False,
                   num_devices=NCORES)
    x_d = nc.dram_tensor("xd", [n_tiles, 128, KC, 128], F32R, kind="ExternalInput")
    wb_d = nc.dram_tensor("wb", [2, 128, KC, HALF], F32R, kind="ExternalInput")
    wd_d = nc.dram_tensor("wd", [MAX_DELTAS, 2, 128, KC, HALF], F32R,
                          kind="ExternalInput")
    out_d = nc.dram_tensor("out", [n_tiles, 128, 2, HALF], F32,
                           kind="ExternalOutput")

    with tile.TileContext(nc) as tc, ExitStack() as ctx:
        xp = ctx.enter_context(tc.tile_pool(name="xp", bufs=2))
        wbp = ctx.enter_context(tc.tile_pool(name="wbp", bufs=1))
        wdp = ctx.enter_context(tc.tile_pool(name="wdp", bufs=2))
        op = ctx.enter_context(tc.tile_pool(name="op", bufs=4))
        pp = ctx.enter_context(tc.tile_pool(name="pp", bufs=6, space="PSUM"))

        for h in range(2):
            wbt = wbp.tile([128, KC * HALF], F32R, tag="wb")
            nc.sync.dma_start(
                wbt[:].rearrange("p (c n) -> p c n", c=KC),
                wb_d.ap()[h])
            wdt = {}
            for ti in range(n_tiles):
                for (g, _, _) in segs[ti]:
                    if g not in wdt:
                        wdt[g] = wdp.tile([128, KC * HALF], F32R, tag="wd",
                                          name=f"wd_{h}_{g}")
                        nc.sync.dma_start(
                            wdt[g][:].rearrange("p (c n) -> p c n", c=KC),
                            wd_d.ap()[g, h])

                xt = xp.tile([128, KC * 128], F32R, tag="x")
                nc.sync.dma_start(
                    xt[:].rearrange("p (c t) -> p c t", c=KC),
                    x_d.ap()[ti])

                ps = pp.tile([128, HALF], F32)
                n_extra = sum(1 for _ in segs[ti])
                for c in range(KC):
                    nc.tensor.matmul(
                        ps[:, :],
                        xt[:, c * 128:(c + 1) * 128],
                        wbt[:, c * HALF:(c + 1) * HALF],
                        start=(c == 0), stop=False,
                        skip_group_check=True)
                for si, (g, r0, r1) in enumerate(segs[ti]):
                    last_seg = si == n_extra - 1
                    for c in range(KC):
                        nc.tensor.matmul(
                            ps[r0:r1, :],
                            xt[:, c * 128 + r0:c * 128 + r1],
                            wdt[g][:, c * HALF:(c + 1) * HALF],
                            start=False, stop=(last_seg and c == KC - 1),
                            skip_group_check=True)

                ot = op.tile([128, HALF], F32)
                nc.scalar.copy(ot[:], ps[:])
                nc.sync.dma_start(out_d.ap()[ti, :, h, :], ot[:])

    nc.compile()
    return nc


def _get_nc(n_tiles, segs):
    key = (n_tiles, tuple(tuple(s) for s in segs))
    if key not in _nc_cache:
        _nc_cache[key] = _build(n_tiles, key[1])
    return _nc_cache[key]


def _unpack_rows(qw):
    # (D, 1, K//PACK, N) int32 -> (D, K, N) 4-bit values, packed along K
    D, _, Kp, N = qw.shape
    shifts = (np.arange(PACK, dtype=np.int32) * 4)
    q = (qw[:, 0, :, None, :] >> shifts[None, None, :, None]) & 0xF
    return q.reshape(D, Kp * PACK, N)


def _unpack_cols(qz):
    # (D, 1, 1, N//PACK) int32 -> (D, N), packed along N
    D = qz.shape[0]
    shifts = (np.arange(PACK, dtype=np.int32) * 4)
    z = (qz[:, 0, 0, :, None] >> shifts[None, None, :]) & 0xF
    return z.reshape(D, -1)


def _dequant(qw, qz, sc):
    q = _unpack_rows(qw).astype(np.float32)
    z = (_unpack_cols(qz) + 1).astype(np.float32)
    return (q - z[:, None, :]) * sc[:, 0, 0, :][:, None, :]


def _prep(inputs):
    x = np.ascontiguousarray(inputs["x"], dtype=np.float32)
    bw = np.asarray(inputs["base_weight"], dtype=np.float32)
    idx = np.asarray(inputs["indices"], dtype=np.int64)

    perm = np.argsort(idx, kind="stable")
    counts = np.bincount(idx, minlength=MAX_DELTAS)
    n_tiles, t_dev, segs, po = _plan(counts)

    # padded-sorted device rows: group g occupies [po[g], po[g]+counts[g])
    dev_rows = np.concatenate(
        [int(po[g]) + np.arange(int(counts[g])) for g in range(MAX_DELTAS)])
    x_pad = np.zeros((t_dev, HIDDEN), dtype=np.float32)
    x_pad[dev_rows] = x[perm]
    # [ti, p, c, t] layout so each token tile is one contiguous 2MB DMA
    x_dev = np.ascontiguousarray(
        x_pad.reshape(n_tiles, 128, KC, 128).transpose(0, 3, 2, 1))

    # per-slice dequant of the int4 deltas (full, then shard columns)
    wd_q = _dequant(np.asarray(inputs["qweight_q"]),
                    np.asarray(inputs["qzeros_q"]),
                    np.asarray(inputs["scales_q"], dtype=np.float32))
    wd_k = _dequant(np.asarray(inputs["qweight_k"]),
                    np.asarray(inputs["qzeros_k"]),
                    np.asarray(inputs["scales_k"], dtype=np.float32))
    wd_v = _dequant(np.asarray(inputs["qweight_v"]),
                    np.asarray(inputs["qzeros_v"]),
                    np.asarray(inputs["scales_v"], dtype=np.float32))

    in_maps = []
    for r in range(NCORES):
        qsl = slice(r * QS, (r + 1) * QS)
        ksl = slice(r * KS, (r + 1) * KS)
        # base shard, K-major: (HIDDEN, NSH)
        rows = np.concatenate([
            np.arange(r * QS, (r + 1) * QS),
            Q_SLICE + np.arange(r * KS, (r + 1) * KS),
            Q_SLICE + KV_SLICE + np.arange(r * KS, (r + 1) * KS)])
        wt = bw[rows].T  # (HIDDEN, NSH)
        wb_dev = np.ascontiguousarray(
            wt.reshape(KC, 128, 2, HALF).transpose(2, 1, 0, 3))
        wd = np.concatenate([wd_q[:, :, qsl], wd_k[:, :, ksl],
                             wd_v[:, :, ksl]], axis=2)  # (D, HIDDEN, NSH)
        wd_dev = np.ascontiguousarray(
            wd.reshape(MAX_DELTAS, KC, 128, 2, HALF).transpose(0, 3, 2, 1, 4))
        in_maps.append({"xd": x_dev, "wb": wb_dev, "wd": wd_dev})
    return in_maps, perm, dev_rows, n_tiles, segs


def _assemble(results, perm, dev_rows):
    outs = [r["out"].reshape(-1, NSH)[dev_rows] for r in results]
    q = np.concatenate([o[:, :QS] for o in outs], axis=1)
    k = np.concatenate([o[:, QS:QS + KS] for o in outs], axis=1)
    v = np.concatenate([o[:, QS + KS:] for o in outs], axis=1)
    out_sorted = np.concatenate([q, k, v], axis=1)
    out = np.empty_like(out_sorted)
    out[perm] = out_sorted
    return out


def run(inputs, trace=False, **kw):
    in_maps, perm, dev_rows, n_tiles, segs = _prep(inputs)
    nc = _get_nc(n_tiles, segs)
    res = bass_utils.run_bass_kernel_spmd(
        nc, in_maps, core_ids=list(range(NCORES)), trace=trace, **kw)
    return _assemble(res.results, perm, dev_rows), res


def kernel(**inputs) -> np.ndarray:
    out, _ = run(inputs)
    return out


# revision 9
# speedup vs baseline: 1.0845x; 1.0845x over previous
"""Trainium2 Bass kernel for MergedQKVParallelLinearWithDelta.

out = x @ base_weight.T + per-token-indexed GPTQ-int4 delta matmul
(out[t] += x[t] @ Wdelta[indices[t]]).

Strategy:
- Tensor-parallel along the output dim N=6144 across 8 cores (768 cols
  each: q 512 + k 128 + v 128), x and indices replicated.
- Host: stable-sort tokens by delta index (MoE routing -> each token
  row is multiplied by exactly one delta, 4x fewer FLOPs than masking),
  transpose x to K-major, dequantize the int4 deltas to fp32 shards.
- Device: per 128-token tile, accumulate base + that tile's delta
  segment(s) into one PSUM bank over 32 K-chunks of float32r matmuls
  (1 cyc/row at N>=256 -> full 78.6 TFLOP/s rate). Output N is split
  into two 384-col halves so weights fit in SBUF.
- Host: concat core shards, unpermute token rows.
"""
import sys

if '/opt/trn_rl_repo' not in sys.path:
    sys.path.insert(0, '/opt/trn_rl_repo')

from contextlib import ExitStack

import numpy as np

import concourse.bass as bass
import concourse.tile as tile
from concourse import bacc, bass_utils, mybir

MAX_DELTAS = 4
PACK = 8
HIDDEN = 4096
Q_SLICE = 4096
KV_SLICE = 1024
TOKENS = 4096
NCORES = 8

QS = Q_SLICE // NCORES          # 512 q cols per core
KS = KV_SLICE // NCORES         # 128 k (and v) cols per core
NSH = QS + 2 * KS               # 768 cols per core
HALF = NSH // 2                 # 384
KC = HIDDEN // 128              # 32 K-chunks
TT = TOKENS // 128              # 32 token tiles

F32R = mybir.dt.float32r
F32 = mybir.dt.float32


def _plan(counts):
    """Pad each delta group to a multiple of 128 tokens so every token
    tile has exactly one delta (full-width matmuls only — PSUM row-offset
    matmuls are ISA-restricted). Returns (n_tiles, t_dev, segs, po)."""
    pc = [(int(c) + 127) // 128 * 128 for c in counts]
    po = np.concatenate([[0], np.cumsum(pc)])
    t_dev = int(po[-1])
    n_tiles = t_dev // 128
    segs = []
    for ti in range(n_tiles):
        t0 = ti * 128
        tile_segs = []
        for g in range(MAX_DELTAS):
            if int(po[g]) <= t0 < int(po[g]) + pc[g] and counts[g] > 0:
                tile_segs.append((g, 0, 128))
        segs.append(tile_segs)
    return n_tiles, t_dev, segs, po


_nc_cache = {}


def _build(n_tiles, segs_key):
    segs = [list(s) for s in segs_key]
    nc = bacc.Bacc("TRN2", target_bir_lowering=False, debug=False,
                   num_devices=NCORES)
    x_d = nc.dram_tensor("xd", [n_tiles, 128, KC, 128], F32R, kind="ExternalInput")
    wb_d = nc.dram_tensor("wb", [2, 128, KC, HALF], F32R, kind="ExternalInput")
    wd_d = nc.dram_tensor("wd", [MAX_DELTAS, 2, 128, KC, HALF], F32R,
                          kind="ExternalInput")
    out_d = nc.dram_tensor("out", [n_tiles, 128, 2, HALF], F32,
                           kind="ExternalOutput")

    SUB = 4                  # weight loads split into SUB sub-tiles
    CPS = KC // SUB          # K-chunks per sub-tile (8)

    with tile.TileContext(nc) as tc, ExitStack() as ctx:
        xp = ctx.enter_context(tc.tile_pool(name="xp", bufs=3))
        wbp = ctx.enter_context(tc.tile_pool(name="wbp", bufs=5))
        wdp = ctx.enter_context(tc.tile_pool(name="wdp", bufs=6))
        op = ctx.enter_context(tc.tile_pool(name="op", bufs=4))
        pp = ctx.enter_context(tc.tile_pool(name="pp", bufs=6, space="PSUM"))

        def load_w(dram_ap, name):
            # one weight matrix as SUB progressive sub-tiles on the ACT
            # HWDGE queue (weights don't block x/out on the SP queue)
            subs = []
            for s in range(SUB):
                t = (wbp if name.startswith("wb") else wdp).tile(
                    [128, CPS * HALF], F32R,
                    tag="wb" if name.startswith("wb") else "wd",
                    name=f"{name}_{s}")
                nc.scalar.dma_start(
                    t[:].rearrange("p (c n) -> p c n", c=CPS),
                    dram_ap[:, s * CPS:(s + 1) * CPS])
                subs.append(t)
            return subs

        def w_chunk(subs, c):
            return subs[c // CPS][:, (c % CPS) * HALF:(c % CPS + 1) * HALF]

        # (h, g) load order; each group's weights prefetched one group early
        group_of_tile = [segs[ti][0][0] if segs[ti] else None
                         for ti in range(n_tiles)]
        load_seq = []
        for h in range(2):
            seen = []
            for ti in range(n_tiles):
                g = group_of_tile[ti]
                if g is not None and (h, g) not in seen:
                    seen.append((h, g))
            load_seq.extend(seen)

        wbt = {}   # h -> subs
        wdt = {}   # (h, g) -> subs
        loaded = 0

        def issue_loads(n):
            # keep n loads in flight beyond what's been consumed
            nonlocal loaded
            while loaded < len(load_seq) and loaded < n:
                h_, g_ = load_seq[loaded]
                if h_ not in wbt:
                    wbt[h_] = load_w(wb_d.ap()[h_], f"wb_{h_}")
                wdt[(h_, g_)] = load_w(wd_d.ap()[g_, h_], f"wd_{h_}_{g_}")
                loaded += 1

        issue_loads(2)
        gi = 0  # index into load_seq of the group being computed
        for h in range(2):
            for ti in range(n_tiles):
                g = group_of_tile[ti]
                if g is not None and load_seq[gi] != (h, g):
                    gi += 1
                    assert load_seq[gi] == (h, g)
                    issue_loads(gi + 2)

                xt = xp.tile([128, KC * 128], F32R, tag="x")
                nc.sync.dma_start(
                    xt[:].rearrange("p (c t) -> p c t", c=KC),
                    x_d.ap()[ti])

                ps = pp.tile([128, HALF], F32)
                n_seg = len(segs[ti])
                for c in range(KC):
                    nc.tensor.matmul(
                        ps[:, :],
                        xt[:, c * 128:(c + 1) * 128],
                        w_chunk(wbt[h], c),
                        start=(c == 0), stop=(n_seg == 0 and c == KC - 1),
                        skip_group_check=True)
                for si, (g_, r0, r1) in enumerate(segs[ti]):
                    for c in range(KC):
                        nc.tensor.matmul(
                            ps[r0:r1, :],
                            xt[:, c * 128 + r0:c * 128 + r1],
                            w_chunk(wdt[(h, g_)], c),
                            start=False,
                            stop=(si == n_seg - 1 and c == KC - 1),
                            skip_group_check=True)

                ot = op.tile([128, HALF], F32)
                nc.scalar.copy(ot[:], ps[:])
                nc.sync.dma_start(out_d.ap()[ti, :, h, :], ot[:])

    nc.compile()
    return nc


def _get_nc(n_tiles, segs):
    key = (n_tiles, tuple(tuple(s) for s in segs))
    if key not in _nc_cache:
        _nc_cache[key] = _build(n_tiles, key[1])
    return _nc_cache[key]


def _unpack_rows(qw):
    # (D, 1, K//PACK, N) int32 -> (D, K, N) 4-bit values, packed along K
    D, _, Kp, N = qw.shape
    shifts = (np.arange(PACK, dtype=np.int32) * 4)
    q = (qw[:, 0, :, None, :] >> shifts[None, None, :, None]) & 0xF
    return q.reshape(D, Kp * PACK, N)


def _unpack_cols(qz):
    # (D, 1, 1, N//PACK) int32 -> (D, N), packed along N
    D = qz.shape[0]
    shifts = (np.arange(PACK, dtype=np.int32) * 4)
    z = (qz[:, 0, 0, :, None] >> shifts[None, None, :]) & 0xF
    return z.reshape(D, -1)


def _dequant(qw, qz, sc):
    q = _unpack_rows(qw).astype(np.float32)
    z = (_unpack_cols(qz) + 1).astype(np.float32)
    return (q - z[:, None, :]) * sc[:, 0, 0, :][:, None, :]


def _prep(inputs):
    x = np.ascontiguousarray(inputs["x"], dtype=np.float32)
    bw = np.asarray(inputs["base_weight"], dtype=np.float32)
    idx = np.asarray(inputs["indices"], dtype=np.int64)

    perm = np.argsort(idx, kind="stable")
    counts = np.bincount(idx, minlength=MAX_DELTAS)
    n_tiles, t_dev, segs, po = _plan(counts)

    # padded-sorted device rows: group g occupies [po[g], po[g]+counts[g])
    dev_rows = np.concatenate(
        [int(po[g]) + np.arange(int(counts[g])) for g in range(MAX_DELTAS)])
    x_pad = np.zeros((t_dev, HIDDEN), dtype=np.float32)
    x_pad[dev_rows] = x[perm]
    # [ti, p, c, t] layout so each token tile is one contiguous 2MB DMA
    x_dev = np.ascontiguousarray(
        x_pad.reshape(n_tiles, 128, KC, 128).transpose(0, 3, 2, 1))

    # per-slice dequant of the int4 deltas (full, then shard columns)
    wd_q = _dequant(np.asarray(inputs["qweight_q"]),
                    np.asarray(inputs["qzeros_q"]),
                    np.asarray(inputs["scales_q"], dtype=np.float32))
    wd_k = _dequant(np.asarray(inputs["qweight_k"]),
                    np.asarray(inputs["qzeros_k"]),
                    np.asarray(inputs["scales_k"], dtype=np.float32))
    wd_v = _dequant(np.asarray(inputs["qweight_v"]),
                    np.asarray(inputs["qzeros_v"]),
                    np.asarray(inputs["scales_v"], dtype=np.float32))

    in_maps = []
    for r in range(NCORES):
        qsl = slice(r * QS, (r + 1) * QS)
        ksl = slice(r * KS, (r + 1) * KS)
        # base shard, K-major: (HIDDEN, NSH)
        rows = np.concatenate([
            np.arange(r * QS, (r + 1) * QS),
            Q_SLICE + np.arange(r * KS, (r + 1) * KS),
            Q_SLICE + KV_SLICE + np.arange(r * KS, (r + 1) * KS)])
        wt = bw[rows].T  # (HIDDEN, NSH)
        wb_dev = np.ascontiguousarray(
            wt.reshape(KC, 128, 2, HALF).transpose(2, 1, 0, 3))
        wd = np.concatenate([wd_q[:, :, qsl], wd_k[:, :, ksl],
                             wd_v[:, :, ksl]], axis=2)  # (D, HIDDEN, NSH)
        wd_dev = np.ascontiguousarray(
            wd.reshape(MAX_DELTAS, KC, 128, 2, HALF).transpose(0, 3, 2, 1, 4))
        in_maps.append({"xd": x_dev, "wb": wb_dev, "wd": wd_dev})
    return in_maps, perm, dev_rows, n_tiles, segs


def _assemble(results, perm, dev_rows):
    outs = [r["out"].reshape(-1, NSH)[dev_rows] for r in results]
    q = np.concatenate([o[:, :QS] for o in outs], axis=1)
    k = np.concatenate([o[:, QS:QS + KS] for o in outs], axis=1)
    v = np.concatenate([o[:, QS + KS:] for o in outs], axis=1)
    out_sorted = np.concatenate([q, k, v], axis=1)
    out = np.empty_like(out_sorted)
    out[perm] = out_sorted
    return out


def run(inputs, trace=False, **kw):
    in_maps, perm, dev_rows, n_tiles, segs = _prep(inputs)
    nc = _get_nc(n_tiles, segs)
    res = bass_utils.run_bass_kernel_spmd(
        nc, in_maps, core_ids=list(range(NCORES)), trace=trace, **kw)
    return _assemble(res.results, perm, dev_rows), res


def kernel(**inputs) -> np.ndarray:
    out, _ = run(inputs)
    return out


# revision 11
# speedup vs baseline: 1.1320x; 1.0438x over previous
"""Trainium2 Bass kernel for MergedQKVParallelLinearWithDelta.

out = x @ base_weight.T + per-token-indexed GPTQ-int4 delta matmul
(out[t] += x[t] @ Wdelta[indices[t]]).

Strategy:
- Tensor-parallel along the output dim N=6144 across 8 cores (768 cols
  each: q 512 + k 128 + v 128), x and indices replicated.
- Host: stable-sort tokens by delta index (MoE routing -> each token
  row is multiplied by exactly one delta, 4x fewer FLOPs than masking),
  transpose x to K-major, dequantize the int4 deltas to fp32 shards.
- Device: per 128-token tile, accumulate base + that tile's delta
  segment(s) into one PSUM bank over 32 K-chunks of float32r matmuls
  (1 cyc/row at N>=256 -> full 78.6 TFLOP/s rate). Output N is split
  into two 384-col halves so weights fit in SBUF.
- Host: concat core shards, unpermute token rows.
"""
import sys

if '/opt/trn_rl_repo' not in sys.path:
    sys.path.insert(0, '/opt/trn_rl_repo')

from contextlib import ExitStack

import numpy as np

import concourse.bass as bass
import concourse.tile as tile
from concourse import bacc, bass_utils, mybir

MAX_DELTAS = 4
PACK = 8
HIDDEN = 4096
Q_SLICE = 4096
KV_SLICE = 1024
TOKENS = 4096
NCORES = 8

QS = Q_SLICE // NCORES          # 512 q cols per core
KS = KV_SLICE // NCORES         # 128 k (and v) cols per core
NSH = QS + 2 * KS               # 768 cols per core
HALF = NSH // 2                 # 384
KC = HIDDEN // 128              # 32 K-chunks
TT = TOKENS // 128              # 32 token tiles

F32R = mybir.dt.float32r
F32 = mybir.dt.float32


def _plan(counts):
    """Pad each delta group to a multiple of 128 tokens so every token
    tile has exactly one delta (full-width matmuls only — PSUM row-offset
    matmuls are ISA-restricted). Returns (n_tiles, t_dev, segs, po)."""
    pc = [(int(c) + 127) // 128 * 128 for c in counts]
    po = np.concatenate([[0], np.cumsum(pc)])
    t_dev = int(po[-1])
    n_tiles = t_dev // 128
    segs = []
    for ti in range(n_tiles):
        t0 = ti * 128
        tile_segs = []
        for g in range(MAX_DELTAS):
            if int(po[g]) <= t0 < int(po[g]) + pc[g] and counts[g] > 0:
                tile_segs.append((g, 0, 128))
        segs.append(tile_segs)
    return n_tiles, t_dev, segs, po


_nc_cache = {}


def _build(n_tiles, segs_key):
    segs = [list(s) for s in segs_key]
    nc = bacc.Bacc("TRN2", target_bir_lowering=False, debug=False,
                   num_devices=NCORES)
    x_d = nc.dram_tensor("xd", [n_tiles, 128, KC, 128], F32R, kind="ExternalInput")
    wb_d = nc.dram_tensor("wb", [2, 128, KC, HALF], F32R, kind="ExternalInput")
    wd_d = nc.dram_tensor("wd", [MAX_DELTAS, 2, 128, KC, HALF], F32R,
                          kind="ExternalInput")
    out_d = nc.dram_tensor("out", [n_tiles, 128, 2, HALF], F32,
                           kind="ExternalOutput")

    SUB = 4                  # weight loads split into SUB sub-tiles
    CPS = KC // SUB          # K-chunks per sub-tile (8)
    SUBX = 2                 # x tiles split into SUBX sub-tiles
    CPX = KC // SUBX         # K-chunks per x sub-tile (16)

    with tile.TileContext(nc) as tc, ExitStack() as ctx:
        xp = ctx.enter_context(tc.tile_pool(name="xp", bufs=5))
        wbp = ctx.enter_context(tc.tile_pool(name="wbp", bufs=4))
        wdp = ctx.enter_context(tc.tile_pool(name="wdp", bufs=8))
        op = ctx.enter_context(tc.tile_pool(name="op", bufs=4))
        pp = ctx.enter_context(tc.tile_pool(name="pp", bufs=6, space="PSUM"))

        def load_w(dram_ap, name):
            # one weight matrix as SUB progressive sub-tiles on the ACT
            # HWDGE queue (weights don't block x/out on the SP queue)
            subs = []
            for s in range(SUB):
                t = (wbp if name.startswith("wb") else wdp).tile(
                    [128, CPS * HALF], F32R,
                    tag="wb" if name.startswith("wb") else "wd",
                    name=f"{name}_{s}")
                nc.scalar.dma_start(
                    t[:].rearrange("p (c n) -> p c n", c=CPS),
                    dram_ap[:, s * CPS:(s + 1) * CPS])
                subs.append(t)
            return subs

        def w_chunk(subs, c):
            return subs[c // CPS][:, (c % CPS) * HALF:(c % CPS + 1) * HALF]

        # (h, g) load order; each group's weights prefetched one group early
        group_of_tile = [segs[ti][0][0] if segs[ti] else None
                         for ti in range(n_tiles)]
        load_seq = []
        for h in range(2):
            seen = []
            for ti in range(n_tiles):
                g = group_of_tile[ti]
                if g is not None and (h, g) not in seen:
                    seen.append((h, g))
            load_seq.extend(seen)

        wbt = {}   # h -> subs
        wdt = {}   # (h, g) -> subs
        loaded = 0

        def issue_loads(n):
            # keep n loads in flight beyond what's been consumed
            nonlocal loaded
            while loaded < len(load_seq) and loaded < n:
                h_, g_ = load_seq[loaded]
                if h_ not in wbt:
                    wbt[h_] = load_w(wb_d.ap()[h_], f"wb_{h_}")
                wdt[(h_, g_)] = load_w(wd_d.ap()[g_, h_], f"wd_{h_}_{g_}")
                loaded += 1

        issue_loads(2)
        gi = 0  # index into load_seq of the group being computed
        for h in range(2):
            for ti in range(n_tiles):
                g = group_of_tile[ti]
                if g is not None and load_seq[gi] != (h, g):
                    gi += 1
                    assert load_seq[gi] == (h, g)
                    issue_loads(gi + 2)

                xts = []
                for s in range(SUBX):
                    xt = xp.tile([128, CPX * 128], F32R, tag="x",
                                 name=f"x_{h}_{ti}_{s}")
                    nc.sync.dma_start(
                        xt[:].rearrange("p (c t) -> p c t", c=CPX),
                        x_d.ap()[ti][:, s * CPX:(s + 1) * CPX])
                    xts.append(xt)

                def x_chunk(c, r0=0, r1=128):
                    t = xts[c // CPX]
                    o = (c % CPX) * 128
                    return t[:, o + r0:o + r1]

                ps = pp.tile([128, HALF], F32)
                n_seg = len(segs[ti])
                for c in range(KC):
                    nc.tensor.matmul(
                        ps[:, :],
                        x_chunk(c),
                        w_chunk(wbt[h], c),
                        start=(c == 0), stop=(n_seg == 0 and c == KC - 1),
                        skip_group_check=True)
                for si, (g_, r0, r1) in enumerate(segs[ti]):
                    for c in range(KC):
                        nc.tensor.matmul(
                            ps[r0:r1, :],
                            x_chunk(c, r0, r1),
                            w_chunk(wdt[(h, g_)], c),
                            start=False,
                            stop=(si == n_seg - 1 and c == KC - 1),
                            skip_group_check=True)

                ot = op.tile([128, HALF], F32)
                nc.scalar.copy(ot[:], ps[:])
                nc.sync.dma_start(out_d.ap()[ti, :, h, :], ot[:])

    nc.compile()
    return nc


def _get_nc(n_tiles, segs):
    key = (n_tiles, tuple(tuple(s) for s in segs))
    if key not in _nc_cache:
        _nc_cache[key] = _build(n_tiles, key[1])
    return _nc_cache[key]


def _unpack_rows(qw):
    # (D, 1, K//PACK, N) int32 -> (D, K, N) 4-bit values, packed along K
    D, _, Kp, N = qw.shape
    shifts = (np.arange(PACK, dtype=np.int32) * 4)
    q = (qw[:, 0, :, None, :] >> shifts[None, None, :, None]) & 0xF
    return q.reshape(D, Kp * PACK, N)


def _unpack_cols(qz):
    # (D, 1, 1, N//PACK) int32 -> (D, N), packed along N
    D = qz.shape[0]
    shifts = (np.arange(PACK, dtype=np.int32) * 4)
    z = (qz[:, 0, 0, :, None] >> shifts[None, None, :]) & 0xF
    return z.reshape(D, -1)


def _dequant(qw, qz, sc):
    q = _unpack_rows(qw).astype(np.float32)
    z = (_unpack_cols(qz) + 1).astype(np.float32)
    return (q - z[:, None, :]) * sc[:, 0, 0, :][:, None, :]


def _prep(inputs):
    x = np.ascontiguousarray(inputs["x"], dtype=np.float32)
    bw = np.asarray(inputs["base_weight"], dtype=np.float32)
    idx = np.asarray(inputs["indices"], dtype=np.int64)

    perm = np.argsort(idx, kind="stable")
    counts = np.bincount(idx, minlength=MAX_DELTAS)
    n_tiles, t_dev, segs, po = _plan(counts)

    # padded-sorted device rows: group g occupies [po[g], po[g]+counts[g])
    dev_rows = np.concatenate(
        [int(po[g]) + np.arange(int(counts[g])) for g in range(MAX_DELTAS)])
    x_pad = np.zeros((t_dev, HIDDEN), dtype=np.float32)
    x_pad[dev_rows] = x[perm]
    # [ti, p, c, t] layout so each token tile is one contiguous 2MB DMA
    x_dev = np.ascontiguousarray(
        x_pad.reshape(n_tiles, 128, KC, 128).transpose(0, 3, 2, 1))

    # per-slice dequant of the int4 deltas (full, then shard columns)
    wd_q = _dequant(np.asarray(inputs["qweight_q"]),
                    np.asarray(inputs["qzeros_q"]),
                    np.asarray(inputs["scales_q"], dtype=np.float32))
    wd_k = _dequant(np.asarray(inputs["qweight_k"]),
                    np.asarray(inputs["qzeros_k"]),
                    np.asarray(inputs["scales_k"], dtype=np.float32))
    wd_v = _dequant(np.asarray(inputs["qweight_v"]),
                    np.asarray(inputs["qzeros_v"]),
                    np.asarray(inputs["scales_v"], dtype=np.float32))

    in_maps = []
    for r in range(NCORES):
        qsl = slice(r * QS, (r + 1) * QS)
        ksl = slice(r * KS, (r + 1) * KS)
        # base shard, K-major: (HIDDEN, NSH)
        rows = np.concatenate([
            np.arange(r * QS, (r + 1) * QS),
            Q_SLICE + np.arange(r * KS, (r + 1) * KS),
            Q_SLICE + KV_SLICE + np.arange(r * KS, (r + 1) * KS)])
        wt = bw[rows].T  # (HIDDEN, NSH)
        wb_dev = np.ascontiguousarray(
            wt.reshape(KC, 128, 2, HALF).transpose(2, 1, 0, 3))
        wd = np.concatenate([wd_q[:, :, qsl], wd_k[:, :, ksl],
                             wd_v[:, :, ksl]], axis=2)  # (D, HIDDEN, NSH)
        wd_dev = np.ascontiguousarray(
            wd.reshape(MAX_DELTAS, KC, 128, 2, HALF).transpose(0, 3, 2, 1, 4))
        in_maps.append({"xd": x_dev, "wb": wb_dev, "wd": wd_dev})
    return in_maps, perm, dev_rows, n_tiles, segs


def _assemble(results, perm, dev_rows):
    outs = [r["out"].reshape(-1, NSH)[dev_rows] for r in results]
    q = np.concatenate([o[:, :QS] for o in outs], axis=1)
    k = np.concatenate([o[:, QS:QS + KS] for o in outs], axis=1)
    v = np.concatenate([o[:, QS + KS:] for o in outs], axis=1)
    out_sorted = np.concatenate([q, k, v], axis=1)
    out = np.empty_like(out_sorted)
    out[perm] = out_sorted
    return out


def run(inputs, trace=False, **kw):
    in_maps, perm, dev_rows, n_tiles, segs = _prep(inputs)
    nc = _get_nc(n_tiles, segs)
    res = bass_utils.run_bass_kernel_spmd(
        nc, in_maps, core_ids=list(range(NCORES)), trace=trace, **kw)
    return _assemble(res.results, perm, dev_rows), res


def kernel(**inputs) -> np.ndarray:
    out, _ = run(inputs)
    return out
